# revision 3
# baseline (speedup 1.0000x reference)
"""Affinity-propagation spatial stencil kernel v5 for Trainium2 (8 NeuronCores).

Data-parallel: 16 images, 2 per core. Per image (H=512, W=512, K=8 gates):

  absw = sum_k shift_k(|G_k|);   gs = sum_k shift_k(G_k)
  inv  = 1/absw = exp(-ln(absw));  bias' = (absw - gs)*raw
  step:  r' = inv * ( sum_k shift_k(G_k * r) + bias' )

Layout (strided): partition p, free dims [c=4, j=512]; image row = 4p + c.
Row shifts: +-1 row = adjacent c-bank view for 3 of 4 banks (identity
matmuls accumulating in PSUM) + one partition-shift matmul per plane.
Column shifts: matmuls read column-offset views of guarded product tiles
(TensorE is alignment-insensitive), keeping every DVE op in aligned
step-1 fp16 2x mode.

v5 changes vs v4:
 - negw stencil planes eliminated: gs stencil streams raw gates through
   the PE (no elementwise planes), bias' = (absw - gs)*raw via two
   ScalarE psum copybacks + DVE sub/mul.
 - step products emitted as 2 half-mega TTs (4 planes each) with a
   stride-0 broadcast view of r: fewer DVE instructions/semaphores.
 - adjacent-bank matmuls merged into N=1024 two-bank MMs (fp16 moving
   operand max is 1024): ~1/3 fewer MM+LDWEIGHTS pairs.
 - image-1 setup |G| planes produced by one 4x-mode mega tensor_scalar.
 - uniform steps (no b0s0 special-casing); bias MMs are emitted last in
   each group and are ready in time.

Engines: DVE products/abs/sub/mul; TensorE shift stencils; ScalarE
ln/exp/copybacks; GPSIMD unused (SBUF port shared with DVE).
"""

import sys

sys.path.insert(0, "/opt/trn_rl_repo")

import numpy as np

import concourse.bass as bass
import concourse.mybir as mybir
from concourse import tile
from concourse.bass_utils import run_bass_kernel_spmd

N_CORES = 8
B, K, H, W = 16, 8, 512, 512
BPC = B // N_CORES
P = 128
C = H // P
GUARD = 2
WG = W + 2 * GUARD
PROP_TIME = 4
OFFSETS = ((1, 1), (1, 0), (1, -1), (0, 1), (0, -1), (-1, 1), (-1, 0), (-1, -1))

F32 = mybir.dt.float32
F16 = mybir.dt.float16
I16 = mybir.dt.int16
AT = mybir.AluOpType
AF = mybir.ActivationFunctionType

W_ID, W_UP, W_DN = range(3)

# feature switches (fallbacks if a HW experiment invalidates one)
USE_MEGA_PRODUCT = True  # broadcast-AP half-mega products
USE_PAIR_MM = False  # N=1024 two-bank matmuls: rejected by walrus ISA check
USE_MEGA_ABS = True  # one 4x TS for image-1 |G| planes

# plane emission order: first plane covers all 4 banks (start flags),
# k=4 (di=0) last before bias. halves: k 0-3 then k 4-7.
PLANE_ORDER = (3, 0, 1, 2, 5, 6, 7, 4)


def make_wmats() -> np.ndarray:
    w = np.zeros((3, P, P), np.float16)
    w[W_ID] = np.eye(P)
    w[W_UP] = np.eye(P, k=-1)  # out[p] = in[p+1]
    w[W_DN] = np.eye(P, k=+1)  # out[p] = in[p-1]
    return w


def _split_excess_waits(nc):
    """This walrus build encodes at most 1 sem wait per instruction; move the
    overflow onto preceding NoOps. Drop EVENT_SEMAPHORE_RANGE_CLEAR."""
    for f in nc.m.functions:
        for bb in f.blocks:
            new_insts = []
            for ins in bb.instructions:
                if getattr(ins, "op_name", None) == "EVENT_SEMAPHORE_RANGE_CLEAR":
                    continue
                cap = 1
                si = getattr(ins, "sync_info", None)
                if si is not None and si.on_wait and len(si.on_wait) > cap:
                    extra = list(si.on_wait[cap:])
                    del si.on_wait[cap:]
                    while extra:
                        nop = mybir.InstNoOp(
                            name=nc.get_next_instruction_name(),
                            engine=ins.engine,
                            sync_info=mybir.SyncInfo(on_wait=extra[:cap], on_update=[]),
                        )
                        new_insts.append(nop)
                        extra = extra[cap:]
                new_insts.append(ins)
            bb.instructions[:] = new_insts


def plane_mms(guarded, k):
    """MM descriptors (wi, rhs, c0, nbanks) for shift-plane k of a guarded
    [P, C, WG] tile. Adjacent-bank identity MMs merged into 2-bank MMs."""
    di, dj = OFFSETS[k]
    v = lambda c0, nb: guarded[:, c0 : c0 + nb, GUARD + dj : GUARD + dj + W]
    if di == 0:
        if USE_PAIR_MM:
            return [(W_ID, v(0, 2), 0, 2), (W_ID, v(2, 2), 2, 2)]
        return [(W_ID, v(c, 1), c, 1) for c in range(C)]
    if di == 1:  # out c <- in c+1; wrap: out 3 = UP @ in 0
        if USE_PAIR_MM:
            return [
                (W_ID, v(1, 2), 0, 2),
                (W_ID, v(3, 1), 2, 1),
                (W_UP, v(0, 1), 3, 1),
            ]
        return [(W_ID, v(c + 1, 1), c, 1) for c in range(C - 1)] + [
            (W_UP, v(0, 1), 3, 1)
        ]
    # di == -1: out c <- in c-1; wrap: out 0 = DN @ in 3
    if USE_PAIR_MM:
        return [
            (W_ID, v(0, 2), 1, 2),
            (W_ID, v(2, 1), 3, 1),
            (W_DN, v(3, 1), 0, 1),
        ]
    return [(W_ID, v(c - 1, 1), c, 1) for c in range(1, C)] + [
        (W_DN, v(3, 1), 0, 1)
    ]


def bias_mms(biasp):
    if USE_PAIR_MM:
        return [(W_ID, biasp[:, 0:2, :], 0, 2), (W_ID, biasp[:, 2:4, :], 2, 2)]
    return [(W_ID, biasp[:, c : c + 1, :], c, 1) for c in range(C)]


def mm_flags(mms):
    """start/stop per MM: start iff first writer of ALL its banks, stop iff
    last. Asserts no partial first/last (guaranteed by plane emission order:
    every plane covers all 4 banks)."""
    first, last = {}, {}
    for i, (_, _, c0, nb) in enumerate(mms):
        for c in range(c0, c0 + nb):
            first.setdefault(c, i)
            last[c] = i
    flags = []
    for i, (_, _, c0, nb) in enumerate(mms):
        banks = range(c0, c0 + nb)
        st = all(first[c] == i for c in banks)
        sp = all(last[c] == i for c in banks)
        assert st or not any(first[c] == i for c in banks), "partial start"
        assert sp or not any(last[c] == i for c in banks), "partial stop"
        flags.append((st, sp))
    return flags


def _in_view(dram_plane):
    return dram_plane.rearrange("(p c) j -> p c j", c=C)


def build(legalize=True, debug=False):
    nc = bass.Bass()
    g_dram = nc.declare_dram_parameter("guidance", [BPC, K, H, W], F16, isOutput=False)
    d_dram = nc.declare_dram_parameter("blur_depth", [BPC, 1, H, W], F16, isOutput=False)
    w_dram = nc.declare_dram_parameter("wmats", [3, P, P], F16, isOutput=False)
    o_dram = nc.declare_dram_parameter("out", [BPC, 1, H, W], F32, isOutput=True)
    if debug:
        dbg = {
            n: nc.declare_dram_parameter(f"dbg_{n}", [BPC, H, W], F32, isOutput=True)
            for n in ("absw", "gs", "inv", "biasp", "r1")
        }

    with tile.TileContext(nc) as tc:
        with (
            tc.tile_pool(name="main", bufs=1) as pool,
            tc.tile_pool(name="stage", bufs=2) as stage_pool,
            tc.tile_pool(name="psum", bufs=2, space="PSUM") as psum_pool,
        ):
            w_sb = pool.tile([P, 3, P], F16, name="w_sb")
            nc.sync.dma_start(out=w_sb[:], in_=w_dram.rearrange("w q p -> q w p"))

            g16 = [pool.tile([P, K, C, WG], F16, name=f"g16_{b}") for b in range(BPC)]
            rt = [pool.tile([P, C, W], F16, name=f"r_{b}") for b in range(BPC)]
            inv = [pool.tile([P, C, W], F16, name=f"inv_{b}") for b in range(BPC)]
            biasp = [pool.tile([P, C, W], F16, name=f"biasp_{b}") for b in range(BPC)]
            # raw16 doubles as the copyback staging (s_sb) once raw is dead
            raw16 = [pool.tile([P, C, W], F16, name=f"raw16_{b}") for b in range(BPC)]
            s_sb = [pool.tile([P, C, W], F16, name=f"s_{b}") for b in range(BPC)]
            # product/abs plane ring: 2 mega slots of 8 planes
            pblk = pool.tile([P, 2, K, C, WG], F16, name="pblk")

            # input DMAs up front; image-0 gates first so setup rides the head
            for b in range(BPC):
                for k in range(K):
                    nc.sync.dma_start(
                        out=g16[b][:, k, :, GUARD : GUARD + W],
                        in_=_in_view(g_dram[b, k]),
                    )
                nc.sync.dma_start(out=raw16[b][:], in_=_in_view(d_dram[b, 0]))
            for b in range(BPC):
                nc.vector.memset(g16[b][:, :, :, 0:GUARD], 0.0)
                nc.vector.memset(g16[b][:, :, :, GUARD + W :], 0.0)
            nc.vector.memset(pblk[:, :, :, :, 0:GUARD], 0.0)
            nc.vector.memset(pblk[:, :, :, :, GUARD + W :], 0.0)

            slot_ctr = [0]

            def ring():
                sl = pblk[:, slot_ctr[0] % 2]
                slot_ctr[0] += 1
                return sl

            def emit(psum, descs, flags):
                for (wi, rhs, c0, nb), (st, sp) in zip(descs, flags):
                    nc.tensor.matmul(
                        psum[:, c0 : c0 + nb, :],
                        w_sb[:, wi, :],
                        rhs,
                        start=st,
                        stop=sp,
                    )

            # ---------------- setup ----------------

            def setup(b):
                """absw + gs stencils -> inv, biasp. Image 0 rides the
                DMA-paced head at per-gate granularity; image 1 (mid-kernel)
                uses one 4x-mode mega |G| op."""
                psa = psum_pool.tile([P, C, W], F32, tag="ps")
                psg = psum_pool.tile([P, C, W], F32, tag="ps")
                slot = ring()
                gv = lambda k: g16[b][:, k]
                abs_descs = [plane_mms(slot[:, k], k) for k in range(K)]
                gs_descs = [plane_mms(gv(k), k) for k in range(K)]
                fa = mm_flags([d for ds in abs_descs for d in ds])
                fg = mm_flags([d for ds in gs_descs for d in ds])
                if b == 1 and USE_MEGA_ABS:
                    nc.vector.tensor_scalar(
                        out=slot[:, :, :, GUARD : GUARD + W].bitcast(I16),
                        in0=g16[b][:, :, :, GUARD : GUARD + W].bitcast(I16),
                        scalar1=0x7FFF,
                        scalar2=None,
                        op0=AT.bitwise_and,
                    )
                ia = ig = 0
                for k in range(K):
                    if not (b == 1 and USE_MEGA_ABS):
                        if k in (1, 5):
                            nc.scalar.activation(
                                slot[:, k, :, GUARD : GUARD + W], gv(k)[
                                    :, :, GUARD : GUARD + W
                                ], AF.Abs,
                            )
                        else:
                            nc.vector.tensor_scalar(
                                out=slot[:, k, :, GUARD : GUARD + W].bitcast(I16),
                                in0=gv(k)[:, :, GUARD : GUARD + W].bitcast(I16),
                                scalar1=0x7FFF,
                                scalar2=None,
                                op0=AT.bitwise_and,
                            )
                    emit(psa, abs_descs[k], fa[ia : ia + len(abs_descs[k])])
                    ia += len(abs_descs[k])
                    emit(psg, gs_descs[k], fg[ig : ig + len(gs_descs[k])])
                    ig += len(gs_descs[k])
                # inv = exp(-ln(absw)); bias' = (absw - gs) * raw
                lnw = stage_pool.tile([P, C, W], F32, tag="stage")
                nc.scalar.activation(lnw[:], psa[:], AF.Ln)
                nc.scalar.activation(inv[b][:], lnw[:], AF.Exp, scale=-1.0)
                nc.scalar.activation(s_sb[b][:], psa[:], AF.Copy)  # absw
                nc.scalar.activation(rt[b][:], psg[:], AF.Copy)  # gs (scratch)
                if debug:
                    d32 = stage_pool.tile([P, C, W], F32, tag="stage")
                    nc.vector.tensor_copy(d32[:], s_sb[b][:])
                    nc.sync.dma_start(out=_in_view(dbg["absw"][b]), in_=d32[:])
                    d32 = stage_pool.tile([P, C, W], F32, tag="stage")
                    nc.vector.tensor_copy(d32[:], rt[b][:])
                    nc.sync.dma_start(out=_in_view(dbg["gs"][b]), in_=d32[:])
                nc.vector.tensor_sub(s_sb[b][:], s_sb[b][:], rt[b][:])
                nc.vector.tensor_mul(biasp[b][:], s_sb[b][:], raw16[b][:])
                if debug:
                    d32 = stage_pool.tile([P, C, W], F32, tag="stage")
                    nc.vector.tensor_copy(d32[:], inv[b][:])
                    nc.sync.dma_start(out=_in_view(dbg["inv"][b]), in_=d32[:])
                    d32 = stage_pool.tile([P, C, W], F32, tag="stage")
                    nc.vector.tensor_copy(d32[:], biasp[b][:])
                    nc.sync.dma_start(out=_in_view(dbg["biasp"][b]), in_=d32[:])

            # ---------------- steps ----------------
            ps_step = [None, None]

            def step_p1(b, step):
                """products (2 half-mega TTs) + shift-stencil + bias -> psum"""
                r_src = raw16[b] if step == 0 else rt[b]
                slot = ring()
                descs = [plane_mms(slot[:, k], k) for k in PLANE_ORDER]
                descs.append(bias_mms(biasp[b]))
                flat = [d for ds in descs for d in ds]
                fl = mm_flags(flat)
                ps = psum_pool.tile([P, C, W], F32, tag="ps")
                for h in (0, 1):
                    if USE_MEGA_PRODUCT:
                        in1 = (
                            r_src[:]
                            .unsqueeze(1)
                            .broadcast_to([P, 4, C, W])
                        )
                        nc.vector.tensor_tensor(
                            slot[:, 4 * h : 4 * h + 4, :, GUARD : GUARD + W],
                            g16[b][:, 4 * h : 4 * h + 4, :, GUARD : GUARD + W],
                            in1,
                            AT.mult,
                        )
                    else:
                        for k in range(4 * h, 4 * h + 4):
                            nc.vector.tensor_mul(
                                slot[:, k, :, GUARD : GUARD + W],
                                g16[b][:, k, :, GUARD : GUARD + W],
                                r_src[:],
                            )
                emit(ps, flat, fl)
                ps_step[b] = ps

            def step_p2(b, step):
                """copyback + renormalize (+ output DMA on last step)"""
                if step == PROP_TIME - 1:
                    out32 = stage_pool.tile([P, C, W], F32, tag="stage")
                    od = _in_view(o_dram[b, 0])
                    for h in range(C):
                        cs = slice(h, h + 1)
                        nc.scalar.activation(
                            s_sb[b][:, cs, :], ps_step[b][:, cs, :], AF.Copy
                        )
                        nc.vector.tensor_mul(
                            out32[:, cs, :], inv[b][:, cs, :], s_sb[b][:, cs, :]
                        )
                        nc.sync.dma_start(out=od[:, cs, :], in_=out32[:, cs, :])
                    return
                nc.scalar.activation(s_sb[b][:], ps_step[b][:], AF.Copy)
                nc.vector.tensor_mul(rt[b][:], inv[b][:], s_sb[b][:])
                if debug and step == 0:
                    d32 = stage_pool.tile([P, C, W], F32, tag="stage")
                    nc.vector.tensor_copy(d32[:], rt[b][:])
                    nc.sync.dma_start(out=_in_view(dbg["r1"][b]), in_=d32[:])

            # ---------------- schedule ----------------
            setup(0)
            step_p1(0, 0)
            step_p2(0, 0)
            setup(1)
            for step in range(1, PROP_TIME):
                step_p1(0, step)
                step_p1(1, step - 1)
                step_p2(0, step)
                step_p2(1, step - 1)
            step_p1(1, PROP_TIME - 1)
            step_p2(1, PROP_TIME - 1)

    if legalize:
        _split_excess_waits(nc)
    return nc


_NC = None


def _get_nc():
    global _NC
    if _NC is None:
        _NC = build()
    return _NC


def run(guidance, blur_depth, **spmd_kwargs):
    nc = _get_nc()
    wm = make_wmats()
    in_maps = [
        {
            "guidance": np.ascontiguousarray(
                guidance[BPC * c : BPC * (c + 1)].astype(np.float16)
            ),
            "blur_depth": np.ascontiguousarray(
                blur_depth[BPC * c : BPC * (c + 1)].astype(np.float16)
            ),
            "wmats": wm,
        }
        for c in range(N_CORES)
    ]
    res = run_bass_kernel_spmd(nc, in_maps, list(range(N_CORES)), **spmd_kwargs)
    out = np.concatenate([res.results[i]["out"] for i in range(N_CORES)], axis=0)
    return out, res


def kernel(guidance, blur_depth):
    out, _ = run(guidance, blur_depth)
    return out.astype(np.float32)


# revision 4
# speedup vs baseline: 1.0166x; 1.0166x over previous
"""Affinity-propagation spatial stencil kernel v6 for Trainium2 (8 NeuronCores).

Data-parallel: 16 images, 2 per core. Per image (H=512, W=512, K=8 gates):

  absw = sum_k shift_k(|G_k|);   gs = sum_k shift_k(G_k)
  inv  = 1/absw = exp(-ln(absw));  bias' = (absw - gs)*raw
  step:  r' = inv * ( sum_k shift_k(G_k * r) + bias' )

Layout (strided): partition p, free dims [c=4, j=512]; image row = 4p + c.
Row shifts: +-1 row = adjacent c-bank view for 3 of 4 banks (identity
matmuls accumulating in PSUM) + one partition-shift matmul per plane.
Column shifts: matmuls read column-offset views of guarded product tiles
(TensorE is alignment-insensitive), keeping every DVE op in aligned
step-1 fp16 2x mode.

v6 vs v4/v5 (HW-measured rationale):
 - negw stencil planes eliminated: the gs stencil streams raw gates
   through the PE (no elementwise planes), bias' = (absw - gs)*raw.
 - step products are 2 half-mega TTs (4 planes each) using a stride-0
   broadcast view of r: ~1us less DVE per step-image + far fewer sems
   (HW: 8-plane broadcast TT = 8.68us, confirmed 2x mode).
 - 3-slot product ring (v5's 2 mega slots stalled products on the
   consuming matmul group; v4's 10 small slots gave ~1.25 steps of
   lookahead, 3 mega slots give 1.5).
 - software-pipelined emission: each inv-mul sits >=1 full product op
   after its matmul group, so the DVE never waits on copybacks.
 - final steps emit their matmul tail bank-by-bank so the per-bank
   copy/mul/DMA chains start while later banks still accumulate.
 - copyback staging aliases the dead raw16 tile; setup scratch uses
   biasp/rt pre-lives (SBUF: 3 mega slots + guarded gates ~ 202KB).

Engines: DVE products/abs/sub/mul; TensorE shift stencils; ScalarE
ln/exp/copybacks; GPSIMD unused (HW probe: its TT crawls 4.1x slower
when DVE runs 2-port ops - DVE starves it on the shared SBUF port).
"""

import sys

sys.path.insert(0, "/opt/trn_rl_repo")

import numpy as np

import concourse.bass as bass
import concourse.mybir as mybir
from concourse import tile
from concourse.bass_utils import run_bass_kernel_spmd

N_CORES = 8
B, K, H, W = 16, 8, 512, 512
BPC = B // N_CORES
P = 128
C = H // P
GUARD = 2
WG = W + 2 * GUARD
PROP_TIME = 4
OFFSETS = ((1, 1), (1, 0), (1, -1), (0, 1), (0, -1), (-1, 1), (-1, 0), (-1, -1))

F32 = mybir.dt.float32
F16 = mybir.dt.float16
I16 = mybir.dt.int16
AT = mybir.AluOpType
AF = mybir.ActivationFunctionType

W_ID, W_UP, W_DN = range(3)
N_SLOT = 3  # product mega-slot ring depth

# plane emission order: first plane covers all 4 banks (start flags);
# halves match the two half-mega product ops (k 0-3, then 4-7).
PLANE_H1 = (3, 0, 1, 2)
PLANE_H2 = (5, 6, 7, 4)


def make_wmats() -> np.ndarray:
    w = np.zeros((3, P, P), np.float16)
    w[W_ID] = np.eye(P)
    w[W_UP] = np.eye(P, k=-1)  # out[p] = in[p+1]
    w[W_DN] = np.eye(P, k=+1)  # out[p] = in[p-1]
    return w


def _split_excess_waits(nc):
    """This walrus build encodes at most 1 sem wait per instruction; move the
    overflow onto preceding NoOps. Drop EVENT_SEMAPHORE_RANGE_CLEAR."""
    for f in nc.m.functions:
        for bb in f.blocks:
            new_insts = []
            for ins in bb.instructions:
                if getattr(ins, "op_name", None) == "EVENT_SEMAPHORE_RANGE_CLEAR":
                    continue
                cap = 1
                si = getattr(ins, "sync_info", None)
                if si is not None and si.on_wait and len(si.on_wait) > cap:
                    extra = list(si.on_wait[cap:])
                    del si.on_wait[cap:]
                    while extra:
                        nop = mybir.InstNoOp(
                            name=nc.get_next_instruction_name(),
                            engine=ins.engine,
                            sync_info=mybir.SyncInfo(on_wait=extra[:cap], on_update=[]),
                        )
                        new_insts.append(nop)
                        extra = extra[cap:]
                new_insts.append(ins)
            bb.instructions[:] = new_insts


def plane_mms(guarded, k, banks=(0, 1, 2, 3)):
    """MM descriptors (wi, rhs, c0) for shift-plane k of a guarded
    [P, C, WG] tile, restricted to the given output banks."""
    di, dj = OFFSETS[k]
    v = lambda c: guarded[:, c : c + 1, GUARD + dj : GUARD + dj + W]
    out = []
    for c in banks:
        if di == 0:
            out.append((W_ID, v(c), c))
        elif di == 1:  # out c <- in c+1; wrap: out 3 = UP @ in 0
            out.append((W_ID, v(c + 1), c) if c < 3 else (W_UP, v(0), 3))
        else:  # out c <- in c-1; wrap: out 0 = DN @ in 3
            out.append((W_ID, v(c - 1), c) if c > 0 else (W_DN, v(3), 0))
    return out


def mm_flags(mms):
    """start/stop per MM: start iff first writer of its bank, stop iff last."""
    first, last = {}, {}
    for i, (_, _, c) in enumerate(mms):
        first.setdefault(c, i)
        last[c] = i
    return [(first[c] == i, last[c] == i) for i, (_, _, c) in enumerate(mms)]


def _in_view(dram_plane):
    return dram_plane.rearrange("(p c) j -> p c j", c=C)


def build(legalize=True, debug=False):
    nc = bass.Bass()
    g_dram = nc.declare_dram_parameter("guidance", [BPC, K, H, W], F16, isOutput=False)
    d_dram = nc.declare_dram_parameter("blur_depth", [BPC, 1, H, W], F16, isOutput=False)
    w_dram = nc.declare_dram_parameter("wmats", [3, P, P], F16, isOutput=False)
    o_dram = nc.declare_dram_parameter("out", [BPC, 1, H, W], F32, isOutput=True)
    if debug:
        dbg = {
            n: nc.declare_dram_parameter(f"dbg_{n}", [BPC, H, W], F32, isOutput=True)
            for n in ("absw", "gs", "inv", "biasp", "r1")
        }

    with tile.TileContext(nc) as tc:
        with (
            tc.tile_pool(name="main", bufs=1) as pool,
            tc.tile_pool(name="stage", bufs=1) as stage_pool,
            tc.tile_pool(name="psum", bufs=2, space="PSUM") as psum_pool,
        ):
            w_sb = pool.tile([P, 3, P], F16, name="w_sb")
            nc.sync.dma_start(out=w_sb[:], in_=w_dram.rearrange("w q p -> q w p"))

            g16 = [pool.tile([P, K, C, WG], F16, name=f"g16_{b}") for b in range(BPC)]
            rt = [pool.tile([P, C, W], F16, name=f"r_{b}") for b in range(BPC)]
            inv = [pool.tile([P, C, W], F16, name=f"inv_{b}") for b in range(BPC)]
            biasp = [pool.tile([P, C, W], F16, name=f"biasp_{b}") for b in range(BPC)]
            # raw16 doubles as the copyback staging once raw is dead
            raw16 = [pool.tile([P, C, W], F16, name=f"raw16_{b}") for b in range(BPC)]
            s_sb = raw16
            pblk = pool.tile([P, N_SLOT, K, C, WG], F16, name="pblk")

            # input DMAs up front; image-0 first so its setup rides the head
            for b in range(BPC):
                for k in range(K):
                    nc.sync.dma_start(
                        out=g16[b][:, k, :, GUARD : GUARD + W],
                        in_=_in_view(g_dram[b, k]),
                    )
                nc.sync.dma_start(out=raw16[b][:], in_=_in_view(d_dram[b, 0]))
            for b in range(BPC):
                nc.vector.memset(g16[b][:, :, :, 0:GUARD], 0.0)
                nc.vector.memset(g16[b][:, :, :, GUARD + W :], 0.0)
            nc.vector.memset(pblk[:, :, :, :, 0:GUARD], 0.0)
            nc.vector.memset(pblk[:, :, :, :, GUARD + W :], 0.0)

            slot_ctr = [0]

            def ring():
                sl = pblk[:, slot_ctr[0] % N_SLOT]
                slot_ctr[0] += 1
                return sl

            def emit(psum, descs, flags):
                for (wi, rhs, c), (st, sp) in zip(descs, flags):
                    nc.tensor.matmul(
                        psum[:, c : c + 1, :], w_sb[:, wi, :], rhs, start=st, stop=sp
                    )

            # ---------------- setup ----------------

            def setup(b):
                """absw + gs stencils -> inv, biasp. Image 0 rides the
                DMA-paced head at per-gate granularity; image 1 (mid-kernel)
                uses one 4x-mode mega |G| op. Scratch: biasp (absw), rt (gs)."""
                psa = psum_pool.tile([P, C, W], F32, tag="ps")
                psg = psum_pool.tile([P, C, W], F32, tag="ps")
                slot = ring()
                abs_descs = [plane_mms(slot[:, k], k) for k in range(K)]
                gs_descs = [plane_mms(g16[b][:, k], k) for k in range(K)]
                fa = mm_flags([d for ds in abs_descs for d in ds])
                fg = mm_flags([d for ds in gs_descs for d in ds])
                if b == 1:
                    nc.vector.tensor_scalar(
                        out=slot[:, :, :, GUARD : GUARD + W].bitcast(I16),
                        in0=g16[b][:, :, :, GUARD : GUARD + W].bitcast(I16),
                        scalar1=0x7FFF,
                        scalar2=None,
                        op0=AT.bitwise_and,
                    )
                ia = ig = 0
                for k in range(K):
                    if b == 0:
                        if k in (1, 5):
                            nc.scalar.activation(
                                slot[:, k, :, GUARD : GUARD + W],
                                g16[b][:, k, :, GUARD : GUARD + W],
                                AF.Abs,
                            )
                        else:
                            nc.vector.tensor_scalar(
                                out=slot[:, k, :, GUARD : GUARD + W].bitcast(I16),
                                in0=g16[b][:, k, :, GUARD : GUARD + W].bitcast(I16),
                                scalar1=0x7FFF,
                                scalar2=None,
                                op0=AT.bitwise_and,
                            )
                    emit(psa, abs_descs[k], fa[ia : ia + 4])
                    ia += 4
                    emit(psg, gs_descs[k], fg[ig : ig + 4])
                    ig += 4
                # inv = exp(-ln(absw)); bias' = (absw - gs) * raw
                lnw = stage_pool.tile([P, C, W], F32, tag="stage")
                nc.scalar.activation(lnw[:], psa[:], AF.Ln)
                nc.scalar.activation(inv[b][:], lnw[:], AF.Exp, scale=-1.0)
                nc.scalar.activation(biasp[b][:], psa[:], AF.Copy)  # absw
                nc.scalar.activation(rt[b][:], psg[:], AF.Copy)  # gs (scratch)
                if debug:
                    d32 = stage_pool.tile([P, C, W], F32, tag="stage")
                    nc.vector.tensor_copy(d32[:], biasp[b][:])
                    nc.sync.dma_start(out=_in_view(dbg["absw"][b]), in_=d32[:])
                    d32 = stage_pool.tile([P, C, W], F32, tag="stage")
                    nc.vector.tensor_copy(d32[:], rt[b][:])
                    nc.sync.dma_start(out=_in_view(dbg["gs"][b]), in_=d32[:])
                nc.vector.tensor_sub(biasp[b][:], biasp[b][:], rt[b][:])
                nc.vector.tensor_mul(biasp[b][:], biasp[b][:], raw16[b][:])
                if debug:
                    d32 = stage_pool.tile([P, C, W], F32, tag="stage")
                    nc.vector.tensor_copy(d32[:], inv[b][:])
                    nc.sync.dma_start(out=_in_view(dbg["inv"][b]), in_=d32[:])
                    d32 = stage_pool.tile([P, C, W], F32, tag="stage")
                    nc.vector.tensor_copy(d32[:], biasp[b][:])
                    nc.sync.dma_start(out=_in_view(dbg["biasp"][b]), in_=d32[:])

            # ---------------- steps ----------------
            ps_step = [None, None]

            def step_p1(b, step):
                """products (2 half-mega TTs) + shift-stencil + bias -> psum.
                Final steps emit the second half bank-by-bank so the per-bank
                output chains can start early."""
                r_src = raw16[b] if step == 0 else rt[b]
                slot = ring()
                final = step == PROP_TIME - 1
                descs = [d for k in PLANE_H1 for d in plane_mms(slot[:, k], k)]
                if final:
                    for c in range(C):
                        for k in PLANE_H2:
                            descs += plane_mms(slot[:, k], k, banks=(c,))
                        descs.append((W_ID, biasp[b][:, c : c + 1, :], c))
                else:
                    descs += [d for k in PLANE_H2 for d in plane_mms(slot[:, k], k)]
                    descs += [
                        (W_ID, biasp[b][:, c : c + 1, :], c) for c in range(C)
                    ]
                fl = mm_flags(descs)
                ps = psum_pool.tile([P, C, W], F32, tag="ps")
                for h in (0, 1):
                    in1 = r_src[:].unsqueeze(1).broadcast_to([P, 4, C, W])
                    nc.vector.tensor_tensor(
                        slot[:, 4 * h : 4 * h + 4, :, GUARD : GUARD + W],
                        g16[b][:, 4 * h : 4 * h + 4, :, GUARD : GUARD + W],
                        in1,
                        AT.mult,
                    )
                emit(ps, descs, fl)
                ps_step[b] = ps

            def step_p2(b, step):
                """copyback + renormalize (+ per-bank output DMA on last step)"""
                if step == PROP_TIME - 1:
                    out32 = stage_pool.tile([P, C, W], F32, tag="stage")
                    od = _in_view(o_dram[b, 0])
                    for h in range(C):
                        cs = slice(h, h + 1)
                        nc.scalar.activation(
                            s_sb[b][:, cs, :], ps_step[b][:, cs, :], AF.Copy
                        )
                        nc.vector.tensor_mul(
                            out32[:, cs, :], inv[b][:, cs, :], s_sb[b][:, cs, :]
                        )
                        nc.sync.dma_start(out=od[:, cs, :], in_=out32[:, cs, :])
                    return
                nc.scalar.activation(s_sb[b][:], ps_step[b][:], AF.Copy)
                nc.vector.tensor_mul(rt[b][:], inv[b][:], s_sb[b][:])
                if debug and step == 0:
                    d32 = stage_pool.tile([P, C, W], F32, tag="stage")
                    nc.vector.tensor_copy(d32[:], rt[b][:])
                    nc.sync.dma_start(out=_in_view(dbg["r1"][b]), in_=d32[:])

            # ---------------- schedule ----------------
            # Software-pipelined: every inv-mul is emitted >= one full
            # half-step of DVE product work after its matmul group, so the
            # DVE queue never blocks on the PE/ScalarE copyback chain.
            setup(0)
            step_p1(0, 0)
            step_p2(0, 0)
            setup(1)
            step_p1(0, 1)
            step_p1(1, 0)
            step_p2(0, 1)
            step_p1(0, 2)
            step_p2(1, 0)
            step_p1(1, 1)
            step_p2(0, 2)
            step_p1(0, 3)
            step_p2(1, 1)
            step_p1(1, 2)
            step_p2(0, 3)
            step_p2(1, 2)
            step_p1(1, 3)
            step_p2(1, 3)

    if legalize:
        _split_excess_waits(nc)
    return nc


_NC = None


def _get_nc():
    global _NC
    if _NC is None:
        _NC = build()
    return _NC


def run(guidance, blur_depth, **spmd_kwargs):
    nc = _get_nc()
    wm = make_wmats()
    in_maps = [
        {
            "guidance": np.ascontiguousarray(
                guidance[BPC * c : BPC * (c + 1)].astype(np.float16)
            ),
            "blur_depth": np.ascontiguousarray(
                blur_depth[BPC * c : BPC * (c + 1)].astype(np.float16)
            ),
            "wmats": wm,
        }
        for c in range(N_CORES)
    ]
    res = run_bass_kernel_spmd(nc, in_maps, list(range(N_CORES)), **spmd_kwargs)
    out = np.concatenate([res.results[i]["out"] for i in range(N_CORES)], axis=0)
    return out, res


def kernel(guidance, blur_depth):
    out, _ = run(guidance, blur_depth)
    return out.astype(np.float32)
